# revision 1
# baseline (speedup 1.0000x reference)
"""Trainium2 Bass kernel for nn_NeuralMemory (retrieve forward pass).

Computes, for x [B, S, D] (flattened to [T, D]):
    q   = x @ wq + bq
    qn  = LayerNorm(q)               (no scale/bias, eps=1e-5)
    h   = qn
    for i in 0..2:  h = silu(h @ mlp_w[i] + mlp_b[i])
    y   = h @ (mlp_w[3] @ w_out) + (mlp_b[3] @ w_out + b_out)
          (layer 3 is linear, so it is folded into w_out on the host;
           the straight-through term is 0 in the forward pass)

Strategy: data-parallel over the 8 NeuronCores (2048 tokens each), all
matmuls in bf16 (1 cyc/row on the PE, end-to-end max rel err ~7e-3 vs
the 2e-2 gate). The host pre-transposes x per core (feature-major) and
pre-casts everything to bf16, so the PE does exactly 5 matmul passes
(q, 3 MLP layers, folded output) = 655k PE cycles (~273 us at 2.4 GHz),
down from 836k for the 6-matmul fp32r version.

Per core: q is computed token-major via the lhsT trick (x^T chunks as
stationary), LayerNorm stats run on the DVE directly from PSUM (1-step
Newton rsqrt), the (q-mu)*rs apply runs on the ACT engine (per-
partition scale/bias) also from PSUM, qn goes feature-major via the
DMA-transpose XBAR (sync queue; measured to land in the standard
kc*128+p layout), the 3 MLP layers contract feature-major with
silu+bias on ACT, and the folded output matmul uses the lhsT trick
again to land token-major for direct DMA out (bf16, upcast on host).
All weights load once (bf16): only the gating wq (+wc, much later)
rides the scalar DGE queue; mw0-2 ride the sync queue BEHIND x so the
startup window moves just 6MB (wc has its own buffer so no WAR-blocked
DMA can be hoisted in front of later queue work); x loads on the sync
queue as kc x token-half chunks (2KB contiguous lines, ~400 GB/s vs
~75 GB/s for 512B lines). A short burst of junk matmuls
warms the PE p-state while the gating x/wq chunks stream in. Measured
~306-308 us (425 us for the fp32r baseline).
"""
from contextlib import ExitStack

import numpy as np
import ml_dtypes

import concourse.bass as bass
import concourse.mybir as mybir
import concourse.tile as tile
from concourse.bass_utils import run_bass_kernel_spmd

D = 1024
P = 128
KC = D // P          # 8 feature chunks of 128
EPS = 1e-5
N_CORES = 8
F32 = mybir.dt.float32
BF = mybir.dt.bfloat16
AF = mybir.ActivationFunctionType
BF_NP = ml_dtypes.bfloat16

# ---------------------------------------------------------------------------
# Walrus in this container accepts at most 1 semaphore wait per instruction.
# Tile emits more; split the extras onto preceding same-engine NOPs (the
# engine executes in order, so waiting on an earlier NOP is equivalent).
MAX_WAITS = 1


def _legalize_waits(nc, max_waits: int = MAX_WAITS) -> int:
    n_split = 0
    for f in nc.m.functions:
        for bb in f.blocks:
            insts = bb.instructions
            new = []
            for inst in insts:
                si = getattr(inst, "sync_info", None)
                waits = list(si.on_wait) if si is not None and si.on_wait else []
                if len(waits) > max_waits:
                    extra, keep = waits[:-max_waits], waits[-max_waits:]
                    for ci in range(0, len(extra), max_waits):
                        chunk = extra[ci:ci + max_waits]
                        nop = mybir.InstNoOp(
                            name=f"{inst.name}-ws{n_split}-{ci}",
                            engine=inst.engine,
                            sync_info=mybir.SyncInfo(on_wait=chunk, on_update=[]),
                            bass_nofuse=True,
                        )
                        new.append(nop)
                    inst.sync_info = mybir.SyncInfo(
                        on_wait=keep, on_update=list(si.on_update or [])
                    )
                    n_split += 1
                new.append(inst)
            if len(new) != len(insts):
                insts[:] = new
    return n_split


# ---------------------------------------------------------------------------
def build_nc(tpc: int = 2048, zero_bq: bool = True, zero_bc: bool = True,
             legalize: bool = True) -> bass.Bass:
    """Per-core kernel: xT [D, tpc] (bf16, feature-major) -> y [tpc, D] bf16."""
    assert tpc % 512 == 0
    TS = tpc // P        # 128-token tiles
    NT = tpc // 512      # 512-token matmul groups
    BLK = 256            # x DMA block (tokens)
    NB = tpc // BLK

    nc = bass.Bass("TRN2", debug=False)

    xT_d = nc.dram_tensor("xT", [D, tpc], BF, kind="ExternalInput").ap()
    wq_d = nc.dram_tensor("wq", [D, D], BF, kind="ExternalInput").ap()
    mw_d = nc.dram_tensor("mw", [3, D, D], BF, kind="ExternalInput").ap()
    wc_d = nc.dram_tensor("wc", [D, D], BF, kind="ExternalInput").ap()
    mb_d = nc.dram_tensor("mb", [3, D], F32, kind="ExternalInput").ap()
    bq_d = nc.dram_tensor("bq", [D], BF, kind="ExternalInput").ap()
    bc_d = nc.dram_tensor("bc", [D], BF, kind="ExternalInput").ap()
    y_d = nc.dram_tensor("y", [tpc, D], BF, kind="ExternalOutput").ap()

    INT32 = mybir.dt.int32
    with tile.TileContext(nc) as tc, ExitStack() as ctx:
        singles = ctx.enter_context(tc.tile_pool(name="singles", bufs=1))
        p_act = ctx.enter_context(tc.tile_pool(name="acts", bufs=3))
        p_w = ctx.enter_context(tc.tile_pool(name="w", bufs=4))
        p_small = ctx.enter_context(tc.tile_pool(name="small", bufs=4))
        p_qn = ctx.enter_context(tc.tile_pool(name="qn", bufs=3))
        p_o = ctx.enter_context(tc.tile_pool(name="o", bufs=2))
        psum = ctx.enter_context(tc.tile_pool(name="ps", bufs=4, space="PSUM"))

        # PE warm-up: junk matmuls on zeroed scratch keep the PE busy from
        # the end of the preamble until the first x/wq chunks land, so the
        # clock is fully ramped when the real chain starts (results unused).
        # Emitted first so the scratch memset is gpsimd's first real op.
        scratch = singles.tile([P, 512], BF, name="scratch")
        nc.gpsimd.memset(scratch[:], 0.0)
        for _ in range(16):
            pjunk = psum.tile([P, 512], F32, name="pq", tag="pq", bufs=6)
            nc.tensor.matmul(pjunk[:], scratch[:, 0:P], scratch[:],
                             start=True, stop=True)

        # --- input DMAs, earliest-needed first ----------------------------
        # Two HW DGE queues run in parallel: x (+ y out) on the sync queue,
        # all weights on the scalar queue. The first x tile and the first
        # kc-half of wq's n0 half gate the first matmul, so they go first
        # and small.
        # x loads as (token-half x kc) chunks: 2KB contiguous lines on both
        # the DRAM and SBUF side (512B-line block loads ran at ~75GB/s and
        # starved phase A; 2KB lines run at ~400GB/s).
        x_sb = p_act.tile([P, KC, tpc], BF, name="x_sb", tag="act")
        xsrc = xT_d.rearrange("(kc p) t -> p kc t", p=P)
        t_half = tpc // 2
        for kc in range(KC):
            nc.sync.dma_start(out=x_sb[:, kc, 0:t_half],
                              in_=xsrc[:, kc, 0:t_half])

        wqsrc = wq_d.rearrange("(kc p) m -> p kc m", p=P)
        w_q = p_w.tile([P, KC, D], BF, name="w_sb", tag="w", bufs=5)
        nc.scalar.dma_start(out=w_q[:, 0:2, 0:512], in_=wqsrc[:, 0:2, 0:512])
        nc.scalar.dma_start(out=w_q[:, 2:4, 0:512], in_=wqsrc[:, 2:4, 0:512])
        nc.scalar.dma_start(out=w_q[:, 4:8, 0:512], in_=wqsrc[:, 4:8, 0:512])
        nc.scalar.dma_start(out=w_q[:, :, 512:1024], in_=wqsrc[:, :, 512:1024])
        for kc in range(KC):
            nc.sync.dma_start(out=x_sb[:, kc, t_half:tpc],
                              in_=xsrc[:, kc, t_half:tpc])
        # mlp weights + wc ride the scalar queue behind wq (not needed until
        # ~95us); x keeps the sync queue to itself so the blocks that gate
        # the phase-A pipeline arrive on time. wc gets its own SBUF buffer
        # (bufs=5) so its load has no WAR wait that could be hoisted ahead
        # of later queue work by the scheduler.
        w_l = []
        for li in range(3):
            w_t = p_w.tile([P, KC, D], BF, name="w_sb", tag="w", bufs=5)
            nc.sync.dma_start(
                out=w_t[:], in_=mw_d[li].rearrange("(kc p) m -> p kc m", p=P)
            )
            w_l.append(w_t)
        w_c = p_w.tile([P, KC, D], BF, name="w_sb", tag="w", bufs=5)
        nc.scalar.dma_start(out=w_c[:], in_=wc_d.rearrange("(kc p) m -> p kc m", p=P))

        # --- constants / biases -------------------------------------------
        magic_t = singles.tile([P, 1], INT32, name="magic_t")
        nc.gpsimd.memset(magic_t[:], 0x5F3759DF)

        mb_sb = singles.tile([P, 3, KC], F32, name="mb_sb")
        nc.gpsimd.dma_start(out=mb_sb[:], in_=mb_d.rearrange("l (mc p) -> p l mc", p=P))

        ones_f32 = singles.tile([1, P], F32, name="ones_f32")
        nc.gpsimd.memset(ones_f32[:], 1.0)
        ones_col = singles.tile([1, P], BF, name="ones_col")
        nc.vector.tensor_copy(ones_col[:], ones_f32[:])
        bq_row = singles.tile([1, D], BF, name="bq_row")
        nc.gpsimd.dma_start(out=bq_row[:], in_=bq_d.rearrange("(a d) -> a d", a=1))
        bc_row = singles.tile([1, D], BF, name="bc_row")
        nc.gpsimd.dma_start(out=bc_row[:], in_=bc_d.rearrange("(a d) -> a d", a=1))

        # --- phase A: q = x @ wq (token-major), LN, transpose to f-major ---
        act0 = p_act.tile([P, KC, tpc], BF, name="act", tag="act")

        def q_group(s, g, pq):
            sl = slice(g * 512, (g + 1) * 512)
            for kc in range(KC):
                nc.tensor.matmul(
                    pq[:], x_sb[:, kc, s * P:(s + 1) * P], w_q[:, kc, sl],
                    start=(kc == 0), stop=(kc == KC - 1 and zero_bq),
                )
            if not zero_bq:
                nc.tensor.matmul(pq[:], ones_col[:], bq_row[:, sl],
                                 start=False, stop=True)

        def stage_ln(st, pq0, pq1, s):  # noqa: ARG001
            nc.vector.bn_stats(out=st[:, 1, :], in_=pq1[:])
            mv = p_small.tile([P, 2], F32, name="mv")
            nc.vector.bn_aggr(out=mv[:], in_=st[:])
            # rsqrt: magic-constant estimate + 2 Newton steps (DVE;
            # keeps sqrt off ACT so the silu tables never reload)
            v_t = p_small.tile([P, 1], F32, name="v_t")
            nc.vector.tensor_scalar_add(out=v_t[:], in0=mv[:, 1:2],
                                        scalar1=float(EPS))
            y_t = p_small.tile([P, 1], F32, name="y_t")
            nc.vector.tensor_scalar(
                out=y_t.bitcast(INT32)[:], in0=v_t.bitcast(INT32)[:],
                scalar1=1, scalar2=None,
                op0=mybir.AluOpType.arith_shift_right,
            )
            nc.vector.tensor_sub(y_t.bitcast(INT32)[:], magic_t[:],
                                 y_t.bitcast(INT32)[:])
            c_t = p_small.tile([P, 1], F32, name="c_t")
            for _ in range(1):
                nc.vector.tensor_mul(c_t[:], y_t[:], y_t[:])
                nc.vector.tensor_mul(c_t[:], c_t[:], v_t[:])
                nc.vector.tensor_scalar(
                    out=c_t[:], in0=c_t[:],
                    scalar1=-0.5, scalar2=1.5,
                    op0=mybir.AluOpType.mult, op1=mybir.AluOpType.add,
                )
                nc.vector.tensor_mul(y_t[:], y_t[:], c_t[:])
            nmurs = p_small.tile([P, 1], F32, name="nmurs")
            nc.vector.tensor_scalar(
                out=nmurs[:], in0=mv[:, 0:1], scalar1=y_t[:], scalar2=-1.0,
                op0=mybir.AluOpType.mult, op1=mybir.AluOpType.mult,
            )
            # qn = (q - mu) * rs == rs * q + (-mu*rs), on ACT from PSUM
            qn = p_qn.tile([P, D], BF, name="qn_tm", tag="qn", bufs=8)
            nc.scalar.activation(out=qn[:, 0:512], in_=pq0[:], func=AF.Identity,
                                 bias=nmurs[:], scale=y_t[:])
            nc.scalar.activation(out=qn[:, 512:1024], in_=pq1[:], func=AF.Identity,
                                 bias=nmurs[:], scale=y_t[:])
            # feature-major via the DMA-transpose XBAR on the sync queue
            # (measured: lands in the standard kc*128+p layout); frees the
            # PE of transposes and ACT of the PSUM copybacks entirely.
            nc.sync.dma_start(out=act0[:, :, s * P:(s + 1) * P], in_=qn[:],
                              transpose=True)
            return qn

        # phase-A pipeline: PE does only the q matmuls; LN stats (DVE),
        # rsqrt (DVE), apply (ACT) and the transpose (DMA XBAR on the sync
        # queue) trail behind; the pq PSUM ring (3 tiles deep) absorbs the
        # chain latency.
        for s in range(TS):
            pq0 = psum.tile([P, 512], F32, name="pq", tag="pq", bufs=6)
            pq1 = psum.tile([P, 512], F32, name="pq", tag="pq", bufs=6)
            q_group(s, 0, pq0)
            st = p_small.tile([P, 2, 6], F32, name="stats")
            nc.vector.bn_stats(out=st[:, 0, :], in_=pq0[:])
            q_group(s, 1, pq1)
            stage_ln(st, pq0, pq1, s)


        # --- phase C: 3 silu MLP layers, feature-major --------------------
        cur = act0
        for li in range(3):
            nxt = p_act.tile([P, KC, tpc], BF, name="act", tag="act")
            w_sb = w_l[li]
            for nt in range(NT):
                tsl = slice(nt * 512, (nt + 1) * 512)
                for mc in range(KC):
                    pm = psum.tile([P, 512], F32, name="pm", tag="pq", bufs=6)
                    for kc in range(KC):
                        nc.tensor.matmul(
                            pm[:], w_sb[:, kc, mc * P:(mc + 1) * P],
                            cur[:, kc, tsl],
                            start=(kc == 0), stop=(kc == KC - 1),
                        )
                    nc.scalar.activation(
                        out=nxt[:, mc, tsl], in_=pm[:],
                        func=AF.Silu, bias=mb_sb[:, li, mc:mc + 1],
                    )
            cur = nxt

        # --- final: y = h @ wc (+ bc), token-major via lhsT trick ----------
        for ts in range(TS):
            o_tm = p_o.tile([P, D], BF, name="o_tm", tag="o")
            for nh in range(2):
                sl = slice(nh * 512, (nh + 1) * 512)
                po = psum.tile([P, 512], F32, name="po", tag="pq", bufs=6)
                for kc in range(KC):
                    nc.tensor.matmul(
                        po[:], cur[:, kc, ts * P:(ts + 1) * P], w_c[:, kc, sl],
                        start=(kc == 0), stop=(kc == KC - 1 and zero_bc),
                    )
                if not zero_bc:
                    nc.tensor.matmul(po[:], ones_col[:], bc_row[:, sl],
                                     start=False, stop=True)
                nc.scalar.copy(o_tm[:, sl], po[:])
            nc.sync.dma_start(out=y_d[ts * P:(ts + 1) * P, :], in_=o_tm[:])

    if legalize:
        _legalize_waits(nc)
    return nc


# ---------------------------------------------------------------------------
_NC_CACHE: dict = {}
TRACE = False
LAST_RESULT = None


def kernel(x, wq, bq, mlp_w, mlp_b, w_out, b_out):
    x = np.asarray(x, dtype=np.float32)
    orig_shape = x.shape
    xf = np.ascontiguousarray(x.reshape(-1, D))
    T = xf.shape[0]
    assert T % N_CORES == 0
    tpc = T // N_CORES

    mlp_w = np.asarray(mlp_w, np.float32)
    mlp_b = np.asarray(mlp_b, np.float32)
    w_out64 = np.asarray(w_out, np.float64)
    wc = (mlp_w[3].astype(np.float64) @ w_out64).astype(np.float32)
    bc = (mlp_b[3].astype(np.float64) @ w_out64
          + np.asarray(b_out, np.float64)).astype(np.float32)
    zero_bq = not np.any(np.asarray(bq))
    zero_bc = not np.any(bc)

    key = (tpc, zero_bq, zero_bc)
    if key not in _NC_CACHE:
        _NC_CACHE[key] = build_nc(tpc, zero_bq, zero_bc)
    nc = _NC_CACHE[key]

    xbf = xf.astype(BF_NP)
    shared = {
        "wq": np.asarray(wq, np.float32).astype(BF_NP),
        "mw": np.ascontiguousarray(mlp_w[:3]).astype(BF_NP),
        "wc": wc.astype(BF_NP),
        "mb": np.ascontiguousarray(mlp_b[:3]),
        "bq": np.asarray(bq, np.float32).astype(BF_NP),
        "bc": bc.astype(BF_NP),
    }
    in_maps = [
        {"xT": np.ascontiguousarray(xbf[c * tpc:(c + 1) * tpc].T), **shared}
        for c in range(N_CORES)
    ]
    try:
        res = run_bass_kernel_spmd(nc, in_maps, list(range(N_CORES)), trace=TRACE)
    except Exception:
        # transient device errors (NRT_EXEC_UNIT_UNRECOVERABLE) recover on retry
        res = run_bass_kernel_spmd(nc, in_maps, list(range(N_CORES)), trace=TRACE)
    global LAST_RESULT
    LAST_RESULT = res
    y = np.concatenate(
        [res.results[c]["y"].astype(np.float32) for c in range(N_CORES)], axis=0
    )
    return y.reshape(orig_shape)



# revision 2
# speedup vs baseline: 1.0034x; 1.0034x over previous
"""Trainium2 Bass kernel for nn_NeuralMemory (retrieve forward pass).

Computes, for x [B, S, D] (flattened to [T, D]):
    q   = x @ wq + bq
    qn  = LayerNorm(q)               (no scale/bias, eps=1e-5)
    h   = qn
    for i in 0..2:  h = silu(h @ mlp_w[i] + mlp_b[i])
    y   = h @ (mlp_w[3] @ w_out) + (mlp_b[3] @ w_out + b_out)
          (layer 3 is linear, so it is folded into w_out on the host;
           the straight-through term is 0 in the forward pass)

Strategy: data-parallel over the 8 NeuronCores (2048 tokens each), all
matmuls in bf16 (1 cyc/row on the PE, end-to-end max rel err ~7e-3 vs
the 2e-2 gate). 5 matmul passes = 655k PE cycles (~276 us at 2.4 GHz
incl. issue overhead); steady state runs at the 216 ns/MM roofline, so
the optimization surface is the startup/tail edges:

- Phase A (q = x@wq, token-major via the lhsT trick) runs in 4 blocks
  of 4 token-tiles with an output-column split (g in {0,1}): block 0
  needs only x[0:512 tokens] (1MB) + wq[:, 0:512] (1MB) before its
  matmuls can run, instead of full wq + half of x (4MB). The host
  packs x and wq so each of those arrives as a single fully-contiguous
  >=0.5MB DMA in exact consumption order (the old 256KB chunked loads
  capped the startup ramp at ~100-240 GB/s; big contiguous DMAs
  sustain ~390 GB/s). This closes the 14-29us PE-starve window that
  also caused a HAM re-throttle (10us at half clock).
- A short burst of junk matmuls covers the ~5us from preamble end to
  first-data arrival so the PE clock is warm when real work starts.
- LayerNorm stats run on the DVE from PSUM (magic-constant rsqrt +
  Newton), the (q-mu)*rs apply on ACT from PSUM, qn goes feature-major
  via the DMA-transpose XBAR on the sync queue.
- 3 MLP layers contract feature-major with silu+bias on ACT; the
  folded output matmul lands token-major via the lhsT trick.
- PSUM is one 8-bank ring shared by all phases (was 6).
- y is stored [2, tpc, 512] so each output half is a single contiguous
  128KB DMA posted right after its ACT copy; the host re-concatenates.
  This shortens the post-last-matmul tail.

Weight queues: x (+ transposes + y) on the sync HWDGE queue; wq halves
then mw0-2 + wc on the scalar HWDGE queue; tiny mb/bq/bc on gpsimd.
"""
from contextlib import ExitStack

import numpy as np
import ml_dtypes

import concourse.bass as bass
import concourse.mybir as mybir
import concourse.tile as tile
from concourse.bass_utils import run_bass_kernel_spmd

D = 1024
P = 128
KC = D // P          # 8 feature chunks of 128
EPS = 1e-5
N_CORES = 8
F32 = mybir.dt.float32
BF = mybir.dt.bfloat16
AF = mybir.ActivationFunctionType
BF_NP = ml_dtypes.bfloat16

N_JUNK = 8           # PE warm-up matmuls (cover preamble->first-data)

# ---------------------------------------------------------------------------
# Walrus in this container accepts at most 1 semaphore wait per instruction.
# Tile emits more; split the extras onto preceding same-engine NOPs (the
# engine executes in order, so waiting on an earlier NOP is equivalent).
MAX_WAITS = 1


def _legalize_waits(nc, max_waits: int = MAX_WAITS) -> int:
    n_split = 0
    for f in nc.m.functions:
        for bb in f.blocks:
            insts = bb.instructions
            new = []
            for inst in insts:
                si = getattr(inst, "sync_info", None)
                waits = list(si.on_wait) if si is not None and si.on_wait else []
                if len(waits) > max_waits:
                    extra, keep = waits[:-max_waits], waits[-max_waits:]
                    for ci in range(0, len(extra), max_waits):
                        chunk = extra[ci:ci + max_waits]
                        nop = mybir.InstNoOp(
                            name=f"{inst.name}-ws{n_split}-{ci}",
                            engine=inst.engine,
                            sync_info=mybir.SyncInfo(on_wait=chunk, on_update=[]),
                            bass_nofuse=True,
                        )
                        new.append(nop)
                    inst.sync_info = mybir.SyncInfo(
                        on_wait=keep, on_update=list(si.on_update or [])
                    )
                    n_split += 1
                new.append(inst)
            if len(new) != len(insts):
                insts[:] = new
    return n_split


# ---------------------------------------------------------------------------
def build_nc(tpc: int = 2048, zero_bq: bool = True, zero_bc: bool = True,
             legalize: bool = True) -> bass.Bass:
    """Per-core kernel: x_d [NB, P, KC, 512] (bf16, host-packed) ->
    y [2, tpc, 512] bf16."""
    assert tpc % 512 == 0
    NB = tpc // 512      # 512-token blocks (phase A blocks / matmul groups)
    BJ = 4               # 128-token tiles per block

    nc = bass.Bass("TRN2", debug=False)

    # host-packed: x_d[q, p, kc, t] = x[q*512+t, kc*128+p]
    x_d = nc.dram_tensor("xq", [NB, P, KC, 512], BF, kind="ExternalInput").ap()
    # host-packed: wq_d[g, p, kc, m] = wq[kc*128+p, g*512+m]
    wq_d = nc.dram_tensor("wq", [2, P, KC, 512], BF, kind="ExternalInput").ap()
    mw_d = nc.dram_tensor("mw", [3, D, D], BF, kind="ExternalInput").ap()
    wc_d = nc.dram_tensor("wc", [D, D], BF, kind="ExternalInput").ap()
    mb_d = nc.dram_tensor("mb", [3, D], F32, kind="ExternalInput").ap()
    bq_d = nc.dram_tensor("bq", [D], BF, kind="ExternalInput").ap()
    bc_d = nc.dram_tensor("bc", [D], BF, kind="ExternalInput").ap()
    y_d = nc.dram_tensor("y", [2, tpc, 512], BF, kind="ExternalOutput").ap()

    INT32 = mybir.dt.int32
    with tile.TileContext(nc) as tc, ExitStack() as ctx:
        singles = ctx.enter_context(tc.tile_pool(name="singles", bufs=1))
        p_act = ctx.enter_context(tc.tile_pool(name="acts", bufs=3))
        p_w = ctx.enter_context(tc.tile_pool(name="w", bufs=5))
        p_small = ctx.enter_context(tc.tile_pool(name="small", bufs=8))
        p_st = ctx.enter_context(tc.tile_pool(name="stats", bufs=4))
        p_qn = ctx.enter_context(tc.tile_pool(name="qn", bufs=8))
        p_o = ctx.enter_context(tc.tile_pool(name="o", bufs=2))
        psum = ctx.enter_context(tc.tile_pool(name="ps", bufs=8, space="PSUM"))

        def ps_tile():
            return psum.tile([P, 512], F32, name="ps", tag="ps", bufs=8)

        # PE warm-up: junk matmuls on zeroed scratch keep the PE busy from
        # the end of the preamble until the first x/wq chunks land, so the
        # clock is fully ramped when the real chain starts (results unused).
        # Emitted first so the scratch memset is gpsimd's first real op.
        scratch = singles.tile([P, 512], BF, name="scratch")
        nc.gpsimd.memset(scratch[:], 0.0)
        for _ in range(N_JUNK):
            pjunk = ps_tile()
            nc.tensor.matmul(pjunk[:], scratch[:, 0:P], scratch[:],
                             start=True, stop=True)

        # --- input DMAs, earliest-needed first ----------------------------
        # sync queue: x blocks (then qn transposes, then y out).
        # scalar queue: wq halves, then mw0-2, wc.
        # Every x/wq load is a single fully-contiguous transfer (host-packed)
        # in exact consumption order; the first two are split in half (kc
        # 0-3 / 4-7) so the very first matmuls can start ~2.5us earlier.
        x_sb = p_act.tile([P, NB, KC, 512], BF, name="x_sb", tag="act")
        nc.sync.dma_start(out=x_sb[:, 0, 0:4, :], in_=x_d[0, :, 0:4, :])
        nc.sync.dma_start(out=x_sb[:, 0, 4:8, :], in_=x_d[0, :, 4:8, :])
        for q in range(1, NB):
            nc.sync.dma_start(out=x_sb[:, q, :, :], in_=x_d[q])

        w_q = p_w.tile([P, 2, KC, 512], BF, name="w_sb", tag="w", bufs=5)
        nc.scalar.dma_start(out=w_q[:, 0, 0:4, :], in_=wq_d[0, :, 0:4, :])
        nc.scalar.dma_start(out=w_q[:, 0, 4:8, :], in_=wq_d[0, :, 4:8, :])
        nc.scalar.dma_start(out=w_q[:, 1, :, :], in_=wq_d[1])
        w_l = []
        for li in range(3):
            w_t = p_w.tile([P, KC, D], BF, name="w_sb", tag="w", bufs=5)
            nc.scalar.dma_start(
                out=w_t[:], in_=mw_d[li].rearrange("(kc p) m -> p kc m", p=P)
            )
            w_l.append(w_t)
        w_c = p_w.tile([P, KC, D], BF, name="w_sb", tag="w", bufs=5)
        nc.scalar.dma_start(out=w_c[:], in_=wc_d.rearrange("(kc p) m -> p kc m", p=P))

        # --- constants / biases -------------------------------------------
        magic_t = singles.tile([P, 1], INT32, name="magic_t")
        nc.gpsimd.memset(magic_t[:], 0x5F3759DF)

        mb_sb = singles.tile([P, 3, KC], F32, name="mb_sb")
        nc.gpsimd.dma_start(out=mb_sb[:], in_=mb_d.rearrange("l (mc p) -> p l mc", p=P))

        ones_f32 = singles.tile([1, P], F32, name="ones_f32")
        nc.gpsimd.memset(ones_f32[:], 1.0)
        ones_col = singles.tile([1, P], BF, name="ones_col")
        nc.vector.tensor_copy(ones_col[:], ones_f32[:])
        bq_row = singles.tile([1, D], BF, name="bq_row")
        nc.gpsimd.dma_start(out=bq_row[:], in_=bq_d.rearrange("(a d) -> a d", a=1))
        bc_row = singles.tile([1, D], BF, name="bc_row")
        nc.gpsimd.dma_start(out=bc_row[:], in_=bc_d.rearrange("(a d) -> a d", a=1))

        # --- phase A: q = x @ wq (token-major), LN, transpose to f-major ---
        act0 = p_act.tile([P, NB, KC, 512], BF, name="act", tag="act")

        def q_group(b, j, g, pq):
            for kc in range(KC):
                nc.tensor.matmul(
                    pq[:], x_sb[:, b, kc, j * P:(j + 1) * P], w_q[:, g, kc, :],
                    start=(kc == 0), stop=(kc == KC - 1 and zero_bq),
                )
            if not zero_bq:
                nc.tensor.matmul(pq[:], ones_col[:],
                                 bq_row[:, g * 512:(g + 1) * 512],
                                 start=False, stop=True)

        def stage_ln(st, pq0, pq1, b, j):
            nc.vector.bn_stats(out=st[:, 1, :], in_=pq1[:])
            mv = p_small.tile([P, 2], F32, name="mv")
            nc.vector.bn_aggr(out=mv[:], in_=st[:])
            # rsqrt: magic-constant estimate + Newton step (DVE;
            # keeps sqrt off ACT so the silu tables never reload)
            v_t = p_small.tile([P, 1], F32, name="v_t")
            nc.vector.tensor_scalar_add(out=v_t[:], in0=mv[:, 1:2],
                                        scalar1=float(EPS))
            y_t = p_small.tile([P, 1], F32, name="y_t")
            nc.vector.tensor_scalar(
                out=y_t.bitcast(INT32)[:], in0=v_t.bitcast(INT32)[:],
                scalar1=1, scalar2=None,
                op0=mybir.AluOpType.arith_shift_right,
            )
            nc.vector.tensor_sub(y_t.bitcast(INT32)[:], magic_t[:],
                                 y_t.bitcast(INT32)[:])
            c_t = p_small.tile([P, 1], F32, name="c_t")
            for _ in range(1):
                nc.vector.tensor_mul(c_t[:], y_t[:], y_t[:])
                nc.vector.tensor_mul(c_t[:], c_t[:], v_t[:])
                nc.vector.tensor_scalar(
                    out=c_t[:], in0=c_t[:],
                    scalar1=-0.5, scalar2=1.5,
                    op0=mybir.AluOpType.mult, op1=mybir.AluOpType.add,
                )
                nc.vector.tensor_mul(y_t[:], y_t[:], c_t[:])
            nmurs = p_small.tile([P, 1], F32, name="nmurs")
            nc.vector.tensor_scalar(
                out=nmurs[:], in0=mv[:, 0:1], scalar1=y_t[:], scalar2=-1.0,
                op0=mybir.AluOpType.mult, op1=mybir.AluOpType.mult,
            )
            # qn = (q - mu) * rs == rs * q + (-mu*rs), on ACT from PSUM
            qn = p_qn.tile([P, D], BF, name="qn_tm", tag="qn", bufs=8)
            nc.scalar.activation(out=qn[:, 0:512], in_=pq0[:], func=AF.Identity,
                                 bias=nmurs[:], scale=y_t[:])
            nc.scalar.activation(out=qn[:, 512:1024], in_=pq1[:], func=AF.Identity,
                                 bias=nmurs[:], scale=y_t[:])
            # feature-major via the DMA-transpose XBAR on the sync queue
            # (lands in the standard kc*128+p layout); frees the PE of
            # transposes and ACT of the PSUM copybacks entirely.
            nc.sync.dma_start(out=act0[:, b, :, j * P:(j + 1) * P], in_=qn[:],
                              transpose=True)

        # Per block b: g=0 matmuls for the 4 token-tiles (needs only x block
        # b + wq half 0), then per tile the g=1 matmuls + LN chain. The four
        # pq0 tiles stay live across the g0 subloop (4 PSUM banks); pq1
        # rotates through the other half of the ring.
        for b in range(NB):
            pq0_l = []
            for j in range(BJ):
                pq0 = ps_tile()
                q_group(b, j, 0, pq0)
                st = p_st.tile([P, 2, 6], F32, name="stats")
                nc.vector.bn_stats(out=st[:, 0, :], in_=pq0[:])
                pq0_l.append((pq0, st))
            for j in range(BJ):
                pq0, st = pq0_l[j]
                pq1 = ps_tile()
                q_group(b, j, 1, pq1)
                stage_ln(st, pq0, pq1, b, j)

        # --- phase C: 3 silu MLP layers, feature-major --------------------
        cur = act0
        for li in range(3):
            nxt = p_act.tile([P, NB, KC, 512], BF, name="act", tag="act")
            w_sb = w_l[li]
            for q in range(NB):
                for mc in range(KC):
                    pm = ps_tile()
                    for kc in range(KC):
                        nc.tensor.matmul(
                            pm[:], w_sb[:, kc, mc * P:(mc + 1) * P],
                            cur[:, q, kc, :],
                            start=(kc == 0), stop=(kc == KC - 1),
                        )
                    nc.scalar.activation(
                        out=nxt[:, q, mc, :], in_=pm[:],
                        func=AF.Silu, bias=mb_sb[:, li, mc:mc + 1],
                    )
            cur = nxt

        # --- final: y = h @ wc (+ bc), token-major via lhsT trick ----------
        # Each 512-col half is copied (ACT) and DMA'd out on its own so the
        # post-last-matmul tail is one 128KB contiguous transfer.
        for ts in range(NB * BJ):
            q, j = ts // BJ, ts % BJ
            o_tm = p_o.tile([P, D], BF, name="o_tm", tag="o")
            for nh in range(2):
                sl = slice(nh * 512, (nh + 1) * 512)
                po = ps_tile()
                for kc in range(KC):
                    nc.tensor.matmul(
                        po[:], cur[:, q, kc, j * P:(j + 1) * P],
                        w_c[:, kc, sl],
                        start=(kc == 0), stop=(kc == KC - 1 and zero_bc),
                    )
                if not zero_bc:
                    nc.tensor.matmul(po[:], ones_col[:], bc_row[:, sl],
                                     start=False, stop=True)
                nc.scalar.copy(o_tm[:, sl], po[:])
                nc.sync.dma_start(out=y_d[nh, ts * P:(ts + 1) * P, :],
                                  in_=o_tm[:, sl])

    if legalize:
        _legalize_waits(nc)
    return nc


# ---------------------------------------------------------------------------
_NC_CACHE: dict = {}
TRACE = False
LAST_RESULT = None


def kernel(x, wq, bq, mlp_w, mlp_b, w_out, b_out):
    x = np.asarray(x, dtype=np.float32)
    orig_shape = x.shape
    xf = np.ascontiguousarray(x.reshape(-1, D))
    T = xf.shape[0]
    assert T % N_CORES == 0
    tpc = T // N_CORES
    NB = tpc // 512

    mlp_w = np.asarray(mlp_w, np.float32)
    mlp_b = np.asarray(mlp_b, np.float32)
    w_out64 = np.asarray(w_out, np.float64)
    wc = (mlp_w[3].astype(np.float64) @ w_out64).astype(np.float32)
    bc = (mlp_b[3].astype(np.float64) @ w_out64
          + np.asarray(b_out, np.float64)).astype(np.float32)
    zero_bq = not np.any(np.asarray(bq))
    zero_bc = not np.any(bc)

    key = (tpc, zero_bq, zero_bc)
    if key not in _NC_CACHE:
        _NC_CACHE[key] = build_nc(tpc, zero_bq, zero_bc)
    nc = _NC_CACHE[key]

    xbf = xf.astype(BF_NP)
    # wq packed as [g, p, kc, m]: wq_d[g, p, kc, m] = wq[kc*128+p, g*512+m]
    wq_bf = np.asarray(wq, np.float32).astype(BF_NP)
    wq_pack = np.ascontiguousarray(
        wq_bf.reshape(KC, P, 2, 512).transpose(2, 1, 0, 3)
    )
    shared = {
        "wq": wq_pack,
        "mw": np.ascontiguousarray(mlp_w[:3]).astype(BF_NP),
        "wc": wc.astype(BF_NP),
        "mb": np.ascontiguousarray(mlp_b[:3]),
        "bq": np.asarray(bq, np.float32).astype(BF_NP),
        "bc": bc.astype(BF_NP),
    }
    # x packed per core as [q, p, kc, t]: x_d[q, p, kc, t] = x[q*512+t, kc*128+p]
    in_maps = []
    for c in range(N_CORES):
        xc = xbf[c * tpc:(c + 1) * tpc]
        xq = np.ascontiguousarray(
            xc.reshape(NB, 512, KC, P).transpose(0, 3, 2, 1)
        )
        in_maps.append({"xq": xq, **shared})
    try:
        res = run_bass_kernel_spmd(nc, in_maps, list(range(N_CORES)), trace=TRACE)
    except Exception:
        # transient device errors (NRT_EXEC_UNIT_UNRECOVERABLE) recover on retry
        res = run_bass_kernel_spmd(nc, in_maps, list(range(N_CORES)), trace=TRACE)
    global LAST_RESULT
    LAST_RESULT = res
    y = np.concatenate(
        [
            np.concatenate(
                [res.results[c]["y"][0], res.results[c]["y"][1]], axis=1
            ).astype(np.float32)
            for c in range(N_CORES)
        ],
        axis=0,
    )
    return y.reshape(orig_shape)
